# revision 29
# baseline (speedup 1.0000x reference)
"""Single-head attention (B=4, N=2048, D=1024), scores scaled by 10.

Sharding: 8 cores = (batch, query-half). Core 2b+h owns queries
[1024h:1024(h+1)] of batch b.

Algebraic restructure: scores = Q K^T = x_q (Wq^T Wk) x_k^T, so
G = q_w^T @ k_w is precomputed on host and the kernel computes
U = x_q G on device — the K projection disappears and the key side of
QK^T is the raw x, which every core receives in full from the host
(no K collective). Only V halves are exchanged, via two pair
AllGathers (one per 512-wide e-half) so the first exchange overlaps
the second half of the V projection.

Numerics: everything runs single-pass fp16 with fp32 PSUM
accumulation (measured end-to-end rel err ~4.6e-3 vs the 2e-2 gate;
the x10 score scale makes bf16 single-pass fail, but fp16's 10-bit
mantissa keeps the softmax exponent error ~0.04). The 1/sum
normalization happens on the HOST: the kernel emits unnormalized O^T
plus a per-query sum row, removing the reciprocal broadcast from the
critical path.

Schedule (fully unrolled, two 512-query chunks): scores stay
k-partitioned (st tiles) so attention@V consumes P with no
transposes. The tensor queue is just proj, QK(0), QK(1), PV(0),
PV(1) plus one rank-1 max-broadcast matmul per chunk. A running
per-query max rides the QK copy stream (one DVE max per k-tile, zero
added latency), is folded 128->32 partitions with 4 small DMAs +
32x32 block transposes to a [1, QCH] row, and each chunk's
scale+exp is interleaved at 2-k-tile / 1-d-tile granularity into the
NEXT tensor block so no engine FIFO ever blocks cross-engine.
Per-query sums: running adds on the otherwise-idle gpsimd engine
(chunk 0) / on vector during PV (chunk 1), finished by one gpsimd
partition_all_reduce each — softmax never touches the tensor engine
for sums. Queue discipline: input loads own the sync ring; V staging
and output writes ride the scalar ring; collectives, V readbacks,
partition reductions, and the sum-row writes stay on the gpsimd
queue (a DMA queued behind a foreign-dependency DMA on the same ring
inherits its wait — keep unrelated consumers on separate rings).
"""

import numpy as np

B, SEQ, D = 4, 2048, 1024
NQ = 1024          # queries per core (= keys computed per core)
QCH = 512          # attention q-chunk
NCH = NQ // QCH    # 2
NCORES = 8
DT = D // 128      # 8 d-tiles
KT = SEQ // 128    # 16 k-tiles
HKT = KT // 2      # 8 own-half k-tiles

_BUILT = {}


def _build():
    if "nc" in _BUILT:
        return _BUILT["nc"]
    from contextlib import ExitStack

    import concourse.bass as bass  # noqa: F401
    import concourse.mybir as mybir
    import concourse.tile as tile
    from concourse import bacc, bass_isa

    dt = mybir.dt
    F32, BF, F16 = dt.float32, dt.bfloat16, dt.float16
    AL = mybir.AluOpType
    EXP = mybir.ActivationFunctionType.Exp
    GROUPS = [[2 * i, 2 * i + 1] for i in range(NCORES // 2)]

    nc = bacc.Bacc("TRN2", target_bir_lowering=False, debug=False)

    xq_d = nc.dram_tensor("xq", [D, NQ], F16, kind="ExternalInput")
    xk_d = nc.dram_tensor("xk", [D, SEQ], F16, kind="ExternalInput")
    g_d = nc.dram_tensor("g", [D, D], F16, kind="ExternalInput")
    wv_d = nc.dram_tensor("wv", [D, D], F16, kind="ExternalInput")
    ot_d = nc.dram_tensor("ot", [D, NQ], F16, kind="ExternalOutput")
    sm_d = nc.dram_tensor("sm", [NCH, QCH], F32, kind="ExternalOutput")

    xq_r = xq_d.ap().rearrange("(t p) n -> p t n", p=128)
    xk_r = xk_d.ap().rearrange("(t p) n -> p t n", p=128)
    g_r = g_d.ap().rearrange("(t p) e -> p t e", p=128)
    wv_r = wv_d.ap().rearrange("(t p) e -> p t e", p=128)
    ot_r = ot_d.ap().rearrange("(t p) q -> p t q", p=128)

    with tile.TileContext(nc) as tc, ExitStack() as ctx:
        main_pool = ctx.enter_context(tc.tile_pool(name="main", bufs=1))
        xk_t = main_pool.tile([128, DT, SEQ], F16, tag="xk")
        uth = main_pool.tile([128, DT, NQ], F16, tag="uth")
        vf = main_pool.tile([128, KT, D], F16, tag="vf")

        const_pool = ctx.enter_context(tc.tile_pool(name="const", bufs=1))
        ten32 = const_pool.tile([1, 128], F32, tag="ten32")
        nc.vector.memset(ten32[:], 10.0)

        dram = ctx.enter_context(tc.tile_pool(name="dram", bufs=1, space="DRAM"))
        v_in0 = dram.tile([NQ, 512], F16, tag="v_in0")
        v_out0 = dram.tile([SEQ, 512], F16, tag="v_out0")
        v_in1 = dram.tile([NQ, 512], F16, tag="v_in1")
        v_out1 = dram.tile([SEQ, 512], F16, tag="v_out1")
        warm_in = dram.tile([16, 16], BF, tag="warm_in")
        warm_out = dram.tile([32, 16], BF, tag="warm_out")

        # tiny warmup collective at t=0: pays the ncfw channel-setup latency
        # before the real V exchanges need it
        warm_sb = const_pool.tile([16, 16], BF, tag="warm_sb")
        nc.vector.memset(warm_sb[:], 0.0)
        nc.sync.dma_start(warm_in[:], warm_sb[:])
        nc.gpsimd.collective_compute(
            "AllGather",
            AL.bypass,
            replica_groups=GROUPS,
            ins=[warm_in[:]],
            outs=[warm_out[:]],
        )

        with (
            tc.tile_pool(name="xqp", bufs=1) as xqp,
            tc.tile_pool(name="gw", bufs=1) as gwpool,
            tc.tile_pool(name="kev", bufs=6) as kevpool,
            tc.tile_pool(name="psA", bufs=4, space="PSUM") as psA,
        ):
            xq_t = xqp.tile([128, DT, NQ], F16, tag="xq")
            wv_t = gwpool.tile([128, DT, D], F16, tag="wv")
            g_t = gwpool.tile([128, DT, D], F16, tag="g")
            # input loads in first-use order, all on the sync ring: the
            # first U-proj psum needs only g's first e-tile + xq h0 (1.25MB)
            nc.sync.dma_start(g_t[:, :, 0:128], g_r[:, :, 0:128])
            for dti in range(DT):
                nc.sync.dma_start(xq_t[:, dti, 0:512], xq_r[:, dti, 0:512])
            nc.sync.dma_start(g_t[:, :, 128:512], g_r[:, :, 128:512])
            nc.sync.dma_start(g_t[:, :, 512:1024], g_r[:, :, 512:1024])
            nc.sync.dma_start(wv_t[:, :, 0:512], wv_r[:, :, 0:512])
            for dti in range(DT):
                nc.sync.dma_start(xq_t[:, dti, 512:1024], xq_r[:, dti, 512:1024])
            nc.sync.dma_start(wv_t[:, :, 512:1024], wv_r[:, :, 512:1024])
            # keys (full x) only needed by attention QK^T — load last
            for j in range(4):
                n0 = 512 * j
                nc.sync.dma_start(xk_t[:, :, n0 : n0 + 512], xk_r[:, :, n0 : n0 + 512])

            def u_proj(chn):
                n0 = 512 * chn
                for et in range(DT):
                    e0 = 128 * et
                    ps = psA.tile([128, 512], F32, tag="psA")
                    for dti in range(DT):
                        nc.tensor.matmul(
                            ps[:],
                            g_t[:, dti, e0 : e0 + 128],
                            xq_t[:, dti, n0 : n0 + 512],
                            start=(dti == 0),
                            stop=(dti == DT - 1),
                        )
                    nc.vector.tensor_copy(uth[:, et, n0 : n0 + 512], ps[:])

            def v_proj(ec, v_in):
                e0 = 512 * ec
                for kt in range(HKT):
                    k0 = 128 * kt
                    ps = psA.tile([128, 512], F32, tag="psA")
                    for dti in range(DT):
                        nc.tensor.matmul(
                            ps[:],
                            xq_t[:, dti, k0 : k0 + 128],
                            wv_t[:, dti, e0 : e0 + 512],
                            start=(dti == 0),
                            stop=(dti == DT - 1),
                        )
                    vev = kevpool.tile([128, 512], F16, tag="vev")
                    nc.vector.tensor_copy(vev[:], ps[:])
                    nc.scalar.dma_start(v_in[k0 : k0 + 128, :], vev[:])

            # ---- U^T chunk 0, then V halves (each launching its exchange),
            # ---- then U^T chunk 1
            u_proj(0)
            for ec, (v_in, v_out) in enumerate(((v_in0, v_out0), (v_in1, v_out1))):
                v_proj(ec, v_in)
                nc.gpsimd.collective_compute(
                    "AllGather",
                    AL.bypass,
                    replica_groups=GROUPS,
                    ins=[v_in[:]],
                    outs=[v_out[:]],
                )
                v_out_r = v_out[:].rearrange("(t p) e -> p t e", p=128)
                e0 = 512 * ec
                for j in range(2):
                    t0 = 8 * j
                    nc.gpsimd.dma_start(
                        vf[:, t0 : t0 + 8, e0 : e0 + 512], v_out_r[:, t0 : t0 + 8, :]
                    )
            u_proj(1)

        # ---------------- Phase B: attention, two 512-query chunks --------
        with (
            tc.tile_pool(name="stp", bufs=2) as stpool,
            tc.tile_pool(name="pp", bufs=2) as ppool,
            tc.tile_pool(name="tree", bufs=1) as treepool,
            tc.tile_pool(name="aux", bufs=2) as auxpool,
            tc.tile_pool(name="osb", bufs=3) as outpool,
            tc.tile_pool(name="psS", bufs=4, space="PSUM") as psS,
            tc.tile_pool(name="psO", bufs=3, space="PSUM") as psO,
            tc.tile_pool(name="psX", bufs=1, space="PSUM") as psX,
        ):
            def qk_tiles(c, st, kts, rmax):
                # QK^T for the given k-tiles; a running per-query max is
                # folded into the copy stream (one DVE max per tile, zero
                # added latency vs a post-hoc reduction)
                q0 = QCH * c
                for kt in kts:
                    k0 = 128 * kt
                    ps = psS.tile([128, QCH], F32, tag="psS")
                    for dti in range(DT):
                        nc.tensor.matmul(
                            ps[:],
                            xk_t[:, dti, k0 : k0 + 128],
                            uth[:, dti, q0 : q0 + QCH],
                            start=(dti == 0),
                            stop=(dti == DT - 1),
                        )
                    nc.vector.tensor_copy(st[:, kt, :], ps[:])
                    if kt == 1:
                        nc.vector.tensor_max(rmax[:], st[:, 0, :], st[:, 1, :])
                    elif kt >= 2:
                        nc.vector.tensor_max(rmax[:], rmax[:], st[:, kt, :])

            def fold_max(rmax):
                # fold the running per-partition max 128 -> 32 partitions
                # (DVE ops need equal start partitions, so move 32-partition
                # groups with DMAs), then 32x32 block transposes down to a
                # [1, QCH] max row
                fold4 = treepool.tile([32, 4, QCH], F32, tag="fold4")
                for a in range(4):
                    nc.sync.dma_start(
                        fold4[:, a, :], rmax[32 * a : 32 * (a + 1), :]
                    )
                nc.vector.tensor_max(fold4[:, 0, :], fold4[:, 0, :], fold4[:, 1, :])
                nc.vector.tensor_max(fold4[:, 2, :], fold4[:, 2, :], fold4[:, 3, :])
                nc.vector.tensor_max(fold4[:, 0, :], fold4[:, 0, :], fold4[:, 2, :])
                t32t = treepool.tile([32, QCH], F32, tag="t32t")
                nc.vector.transpose(t32t[:], fold4[:, 0, :])
                mx32 = treepool.tile([32, 32], F32, tag="mx32")
                nc.vector.memset(mx32[:], 0.0)
                nc.vector.reduce_max(
                    mx32[:, 0 : QCH // 32],
                    t32t[:].rearrange("p (j c) -> p j c", c=32),
                    axis=mybir.AxisListType.X,
                )
                mx32t = treepool.tile([32, 32], F32, tag="mx32t")
                nc.vector.transpose(mx32t[:], mx32[:])
                m1row = treepool.tile([1, QCH], F32, tag="m1row")
                nc.sync.dma_start(m1row[:], mx32t[0 : QCH // 32, :])
                return m1row

            def bcast_max(m1row):
                # broadcast 10*max across partitions with a rank-1 matmul
                maxb_ps = psX.tile([128, QCH], F32, tag="bcast")
                nc.tensor.matmul(
                    maxb_ps[:], ten32[:], m1row[:], start=True, stop=True
                )
                maxb = auxpool.tile([128, QCH], F32, tag="maxb")
                nc.vector.tensor_copy(maxb[:], maxb_ps[:])
                return maxb

            def stt_exp(st, maxb, p_t, j, rsum):
                # exp(10*s - 10*max) for k-tiles 4j..4j+3; one wide exp
                # amortizes the ~185ns activation launch overhead. The
                # otherwise-idle gpsimd engine accumulates the running
                # per-query sum of P behind each exp batch.
                for kt in range(4 * j, 4 * j + 4):
                    nc.vector.scalar_tensor_tensor(
                        st[:, kt, :],
                        st[:, kt, :],
                        10.0,
                        maxb[:],
                        op0=AL.mult,
                        op1=AL.subtract,
                    )
                nc.scalar.activation(
                    p_t[:, 4 * j : 4 * j + 4, :],
                    st[:, 4 * j : 4 * j + 4, :],
                    EXP,
                )
                if rsum is None:
                    return
                for kt in range(4 * j, 4 * j + 4):
                    if kt == 0:
                        continue
                    if kt == 1:
                        nc.gpsimd.tensor_add(
                            rsum[:], p_t[:, 0, :], p_t[:, 1, :]
                        )
                    else:
                        nc.gpsimd.tensor_add(rsum[:], rsum[:], p_t[:, kt, :])

            def pv_tiles(c, p_t, dtis):
                q0 = QCH * c
                for dti in dtis:
                    d0 = 128 * dti
                    ops = psO.tile([128, QCH], F32, tag="psO")
                    for kt in range(KT):
                        nc.tensor.matmul(
                            ops[:],
                            vf[:, kt, d0 : d0 + 128],
                            p_t[:, kt, :],
                            start=(kt == 0),
                            stop=(kt == KT - 1),
                        )
                    osb = outpool.tile([128, QCH], F16, tag="osb")
                    nc.vector.tensor_copy(osb[:], ops[:])
                    nc.scalar.dma_start(ot_r[:, dti, q0 : q0 + QCH], osb[:])

            def sum_row(c, rsum):
                # finish the per-query sum across the 128 partitions with a
                # gpsimd all-reduce; row 0 ships to host for the 1/sum
                # normalization. Never touches the tensor or vector engines;
                # nothing downstream depends on it.
                sall = auxpool.tile([128, QCH], F32, tag="sall")
                nc.gpsimd.partition_all_reduce(
                    sall[:], rsum[:], 128, bass_isa.ReduceOp.add
                )
                nc.gpsimd.dma_start(sm_d.ap()[c : c + 1, :], sall[0:1, :])

            # unrolled schedule (NCH == 2): each chunk's softmax (STT/exp) is
            # interleaved at fine grain into the next tensor tile stream
            # (chunk 1's QK / chunk 0's PV) so no engine FIFO ever blocks on
            # a cross-engine dependency
            st0 = stpool.tile([128, KT, QCH], F32, tag="st")
            rmax0 = auxpool.tile([128, QCH], F32, tag="rmax")
            qk_tiles(0, st0, range(KT), rmax0)
            m1row0 = fold_max(rmax0)
            st1 = stpool.tile([128, KT, QCH], F32, tag="st")
            rmax1 = auxpool.tile([128, QCH], F32, tag="rmax")
            qk_tiles(1, st1, range(0, 5), rmax1)
            maxb0 = bcast_max(m1row0)
            p0 = ppool.tile([128, KT, QCH], F16, tag="p")
            rsum0 = auxpool.tile([128, QCH], F32, tag="rsum")
            for j in range(4):
                qk_tiles(1, st1, range(5 + 2 * j, 7 + 2 * j), rmax1)
                stt_exp(st0, maxb0, p0, j, rsum0)
            qk_tiles(1, st1, range(13, KT), rmax1)
            m1row1 = fold_max(rmax1)
            pv_tiles(0, p0, range(0, 3))
            sum_row(0, rsum0)
            maxb1 = bcast_max(m1row1)
            p1 = ppool.tile([128, KT, QCH], F16, tag="p")
            for j in range(4):
                stt_exp(st1, maxb1, p1, j, None)
                pv_tiles(0, p0, range(3 + j, 4 + j))
            pv_tiles(0, p0, range(7, DT))
            # chunk 1's sums run on the vector engine (idle during PV) so
            # the serial gpsimd add chain doesn't extend the kernel tail
            rsum1 = auxpool.tile([128, QCH], F32, tag="rsum")
            nc.vector.tensor_add(rsum1[:], p1[:, 0, :], p1[:, 1, :])
            for kt in range(2, KT):
                nc.vector.tensor_add(rsum1[:], rsum1[:], p1[:, kt, :])
            sum_row(1, rsum1)
            pv_tiles(1, p1, range(DT))

    nc.compile()
    _BUILT["nc"] = nc
    return nc


def _prep_inputs(x, q_w, k_w, v_w):
    f16 = np.float16
    g = np.ascontiguousarray(q_w.T @ k_w).astype(f16)
    wv = np.ascontiguousarray(v_w.T).astype(f16)

    in_maps = []
    xk_cache = {}
    for core in range(NCORES):
        b, h = divmod(core, 2)
        if b not in xk_cache:
            xk_cache[b] = np.ascontiguousarray(np.asarray(x[b]).T).astype(f16)
        xk = xk_cache[b]
        xq = np.ascontiguousarray(xk[:, NQ * h : NQ * (h + 1)])
        in_maps.append({"xq": xq, "xk": xk, "g": g, "wv": wv})
    return in_maps


def run(x, q_w, k_w, v_w, trace=False):
    from concourse.bass_utils import run_bass_kernel_spmd

    nc = _build()
    in_maps = _prep_inputs(x, q_w, k_w, v_w)
    res = run_bass_kernel_spmd(nc, in_maps, list(range(NCORES)), trace=trace)
    out = np.empty((B, SEQ, D), np.float32)
    for core in range(NCORES):
        b, h = divmod(core, 2)
        ot = res.results[core]["ot"].T.astype(np.float32)
        sm = res.results[core]["sm"].reshape(NQ).astype(np.float32)
        out[b, NQ * h : NQ * (h + 1)] = ot / sm[:, None]
    return out, res


def kernel(x, q_w, k_w, v_w):
    x = np.asarray(x, np.float32)
    q_w = np.asarray(q_w, np.float32)
    k_w = np.asarray(k_w, np.float32)
    v_w = np.asarray(v_w, np.float32)
    out, _ = run(x, q_w, k_w, v_w, trace=False)
    return out


# revision 31
# speedup vs baseline: 1.0186x; 1.0186x over previous
"""Single-head attention (B=4, N=2048, D=1024), scores scaled by 10.

Sharding: 8 cores = (batch, query-half). Core 2b+h owns queries
[1024h:1024(h+1)] of batch b.

Algebraic restructure: scores = Q K^T = x_q (Wq^T Wk) x_k^T, so
G = q_w^T @ k_w is precomputed on host and the kernel computes
U = x_q G on device — the K projection disappears and the key side of
QK^T is the raw x, which every core receives in full from the host
(no K collective). Only V halves are exchanged, via two pair
AllGathers (one per 512-wide e-half) so the first exchange overlaps
the second half of the V projection.

Numerics: everything runs single-pass fp16 with fp32 PSUM
accumulation (measured end-to-end rel err ~4.6e-3 vs the 2e-2 gate;
the x10 score scale makes bf16 single-pass fail, but fp16's 10-bit
mantissa keeps the softmax exponent error ~0.04). The 1/sum
normalization happens on the HOST: the kernel emits unnormalized O^T
plus a per-query sum row, removing the reciprocal broadcast from the
critical path.

Schedule (fully unrolled, two 512-query chunks): scores stay
k-partitioned (st tiles) so attention@V consumes P with no
transposes. The tensor queue is just proj, QK(0), QK(1), PV(0),
PV(1) plus one rank-1 max-broadcast matmul per chunk. A running
per-query max rides the QK copy stream (one DVE max per k-tile, zero
added latency), is folded 128->32 partitions with 4 small DMAs +
32x32 block transposes to a [1, QCH] row, and each chunk's scale+exp
is interleaved at 2-k-tile / 1-d-tile granularity into the NEXT
tensor block so no engine FIFO ever blocks cross-engine. Per-query
sums: running adds on the otherwise-idle gpsimd engine (chunk 0) /
on vector during PV (chunk 1), each finished by one gpsimd
partition_all_reduce — sums never touch the tensor engine. Queue
discipline: input loads own the sync ring; V staging and output
writes ride the scalar ring; collectives, V readbacks, partition
reductions, and the sum-row writes stay on the gpsimd queue (a DMA
queued behind a foreign-dependency DMA on the same ring inherits its
wait — keep unrelated consumers on separate rings).
"""

import numpy as np

B, SEQ, D = 4, 2048, 1024
NQ = 1024          # queries per core (= keys computed per core)
QCH = 512          # attention q-chunk
NCH = NQ // QCH    # 2
NCORES = 8
DT = D // 128      # 8 d-tiles
KT = SEQ // 128    # 16 k-tiles
HKT = KT // 2      # 8 own-half k-tiles

_BUILT = {}


def _build():
    if "nc" in _BUILT:
        return _BUILT["nc"]
    from contextlib import ExitStack

    import concourse.bass as bass  # noqa: F401
    import concourse.mybir as mybir
    import concourse.tile as tile
    from concourse import bacc, bass_isa

    dt = mybir.dt
    F32, BF, F16 = dt.float32, dt.bfloat16, dt.float16
    AL = mybir.AluOpType
    EXP = mybir.ActivationFunctionType.Exp
    GROUPS = [[2 * i, 2 * i + 1] for i in range(NCORES // 2)]

    nc = bacc.Bacc("TRN2", target_bir_lowering=False, debug=False)

    xq_d = nc.dram_tensor("xq", [D, NQ], F16, kind="ExternalInput")
    xk_d = nc.dram_tensor("xk", [D, SEQ], F16, kind="ExternalInput")
    g_d = nc.dram_tensor("g", [D, D], F16, kind="ExternalInput")
    wv_d = nc.dram_tensor("wv", [D, D], F16, kind="ExternalInput")
    ot_d = nc.dram_tensor("ot", [D, NQ], F16, kind="ExternalOutput")
    sm_d = nc.dram_tensor("sm", [NCH, QCH], F32, kind="ExternalOutput")

    xq_r = xq_d.ap().rearrange("(t p) n -> p t n", p=128)
    xk_r = xk_d.ap().rearrange("(t p) n -> p t n", p=128)
    g_r = g_d.ap().rearrange("(t p) e -> p t e", p=128)
    wv_r = wv_d.ap().rearrange("(t p) e -> p t e", p=128)
    ot_r = ot_d.ap().rearrange("(t p) q -> p t q", p=128)

    with tile.TileContext(nc) as tc, ExitStack() as ctx:
        main_pool = ctx.enter_context(tc.tile_pool(name="main", bufs=1))
        xk_t = main_pool.tile([128, DT, SEQ], F16, tag="xk")
        uth = main_pool.tile([128, DT, NQ], F16, tag="uth")
        vf = main_pool.tile([128, KT, D], F16, tag="vf")

        const_pool = ctx.enter_context(tc.tile_pool(name="const", bufs=1))
        ten32 = const_pool.tile([1, 128], F32, tag="ten32")
        nc.vector.memset(ten32[:], 10.0)

        dram = ctx.enter_context(tc.tile_pool(name="dram", bufs=1, space="DRAM"))
        v_in0 = dram.tile([NQ, 512], F16, tag="v_in0")
        v_out0 = dram.tile([SEQ, 512], F16, tag="v_out0")
        v_in1 = dram.tile([NQ, 512], F16, tag="v_in1")
        v_out1 = dram.tile([SEQ, 512], F16, tag="v_out1")
        warm_in = dram.tile([16, 16], BF, tag="warm_in")
        warm_out = dram.tile([32, 16], BF, tag="warm_out")

        # tiny warmup collective at t=0: pays the ncfw channel-setup latency
        # before the real V exchanges need it
        warm_sb = const_pool.tile([16, 16], BF, tag="warm_sb")
        nc.vector.memset(warm_sb[:], 0.0)
        nc.sync.dma_start(warm_in[:], warm_sb[:])
        nc.gpsimd.collective_compute(
            "AllGather",
            AL.bypass,
            replica_groups=GROUPS,
            ins=[warm_in[:]],
            outs=[warm_out[:]],
        )

        with (
            tc.tile_pool(name="xqp", bufs=1) as xqp,
            tc.tile_pool(name="gw", bufs=1) as gwpool,
            tc.tile_pool(name="kev", bufs=6) as kevpool,
            tc.tile_pool(name="psA", bufs=4, space="PSUM") as psA,
        ):
            xq_t = xqp.tile([128, DT, NQ], F16, tag="xq")
            wv_t = gwpool.tile([128, DT, D], F16, tag="wv")
            g_t = gwpool.tile([128, DT, D], F16, tag="g")
            # input loads in first-use order, all on the sync ring:
            # U-proj chunk 0 needs g h0 + xq h0 (2MB) — tensor starts ~+12us
            nc.sync.dma_start(g_t[:, :, 0:512], g_r[:, :, 0:512])
            for dti in range(DT):
                nc.sync.dma_start(xq_t[:, dti, 0:512], xq_r[:, dti, 0:512])
            nc.sync.dma_start(g_t[:, :, 512:1024], g_r[:, :, 512:1024])
            nc.sync.dma_start(wv_t[:, :, 0:512], wv_r[:, :, 0:512])
            for dti in range(DT):
                nc.sync.dma_start(xq_t[:, dti, 512:1024], xq_r[:, dti, 512:1024])
            nc.sync.dma_start(wv_t[:, :, 512:1024], wv_r[:, :, 512:1024])
            # keys (full x) only needed by attention QK^T — load last
            for j in range(4):
                n0 = 512 * j
                nc.sync.dma_start(xk_t[:, :, n0 : n0 + 512], xk_r[:, :, n0 : n0 + 512])

            def u_proj(chn):
                n0 = 512 * chn
                for et in range(DT):
                    e0 = 128 * et
                    ps = psA.tile([128, 512], F32, tag="psA")
                    for dti in range(DT):
                        nc.tensor.matmul(
                            ps[:],
                            g_t[:, dti, e0 : e0 + 128],
                            xq_t[:, dti, n0 : n0 + 512],
                            start=(dti == 0),
                            stop=(dti == DT - 1),
                        )
                    nc.vector.tensor_copy(uth[:, et, n0 : n0 + 512], ps[:])

            def v_proj(ec, v_in):
                e0 = 512 * ec
                for kt in range(HKT):
                    k0 = 128 * kt
                    ps = psA.tile([128, 512], F32, tag="psA")
                    for dti in range(DT):
                        nc.tensor.matmul(
                            ps[:],
                            xq_t[:, dti, k0 : k0 + 128],
                            wv_t[:, dti, e0 : e0 + 512],
                            start=(dti == 0),
                            stop=(dti == DT - 1),
                        )
                    vev = kevpool.tile([128, 512], F16, tag="vev")
                    nc.vector.tensor_copy(vev[:], ps[:])
                    nc.scalar.dma_start(v_in[k0 : k0 + 128, :], vev[:])

            # ---- U^T chunk 0, then V halves (each launching its exchange),
            # ---- then U^T chunk 1
            u_proj(0)
            for ec, (v_in, v_out) in enumerate(((v_in0, v_out0), (v_in1, v_out1))):
                v_proj(ec, v_in)
                nc.gpsimd.collective_compute(
                    "AllGather",
                    AL.bypass,
                    replica_groups=GROUPS,
                    ins=[v_in[:]],
                    outs=[v_out[:]],
                )
                v_out_r = v_out[:].rearrange("(t p) e -> p t e", p=128)
                e0 = 512 * ec
                for j in range(2):
                    t0 = 8 * j
                    nc.gpsimd.dma_start(
                        vf[:, t0 : t0 + 8, e0 : e0 + 512], v_out_r[:, t0 : t0 + 8, :]
                    )
            u_proj(1)

        # ---------------- Phase B: attention, two 512-query chunks --------
        with (
            tc.tile_pool(name="stp", bufs=2) as stpool,
            tc.tile_pool(name="pp", bufs=2) as ppool,
            tc.tile_pool(name="tree", bufs=1) as treepool,
            tc.tile_pool(name="aux", bufs=2) as auxpool,
            tc.tile_pool(name="osb", bufs=3) as outpool,
            tc.tile_pool(name="psS", bufs=4, space="PSUM") as psS,
            tc.tile_pool(name="psO", bufs=3, space="PSUM") as psO,
            tc.tile_pool(name="psX", bufs=1, space="PSUM") as psX,
        ):
            def qk_tiles(c, st, kts, rmax):
                # QK^T for the given k-tiles; a running per-query max is
                # folded into the copy stream (one DVE max per tile, zero
                # added latency vs a post-hoc reduction)
                q0 = QCH * c
                for kt in kts:
                    k0 = 128 * kt
                    ps = psS.tile([128, QCH], F32, tag="psS")
                    for dti in range(DT):
                        nc.tensor.matmul(
                            ps[:],
                            xk_t[:, dti, k0 : k0 + 128],
                            uth[:, dti, q0 : q0 + QCH],
                            start=(dti == 0),
                            stop=(dti == DT - 1),
                        )
                    nc.vector.tensor_copy(st[:, kt, :], ps[:])
                    if kt == 1:
                        nc.vector.tensor_max(rmax[:], st[:, 0, :], st[:, 1, :])
                    elif kt >= 2:
                        nc.vector.tensor_max(rmax[:], rmax[:], st[:, kt, :])

            def fold_max(rmax):
                # fold the running per-partition max 128 -> 32 partitions
                # (DVE ops need equal start partitions, so move 32-partition
                # groups with DMAs), then 32x32 block transposes down to a
                # [1, QCH] max row
                fold4 = treepool.tile([32, 4, QCH], F32, tag="fold4")
                for a in range(4):
                    nc.sync.dma_start(
                        fold4[:, a, :], rmax[32 * a : 32 * (a + 1), :]
                    )
                nc.vector.tensor_max(fold4[:, 0, :], fold4[:, 0, :], fold4[:, 1, :])
                nc.vector.tensor_max(fold4[:, 2, :], fold4[:, 2, :], fold4[:, 3, :])
                nc.vector.tensor_max(fold4[:, 0, :], fold4[:, 0, :], fold4[:, 2, :])
                t32t = treepool.tile([32, QCH], F32, tag="t32t")
                nc.vector.transpose(t32t[:], fold4[:, 0, :])
                mx32 = treepool.tile([32, 32], F32, tag="mx32")
                nc.vector.memset(mx32[:], 0.0)
                nc.vector.reduce_max(
                    mx32[:, 0 : QCH // 32],
                    t32t[:].rearrange("p (j c) -> p j c", c=32),
                    axis=mybir.AxisListType.X,
                )
                mx32t = treepool.tile([32, 32], F32, tag="mx32t")
                nc.vector.transpose(mx32t[:], mx32[:])
                m1row = treepool.tile([1, QCH], F32, tag="m1row")
                nc.sync.dma_start(m1row[:], mx32t[0 : QCH // 32, :])
                return m1row

            def bcast_max(m1row):
                # broadcast 10*max across partitions with a rank-1 matmul
                maxb_ps = psX.tile([128, QCH], F32, tag="bcast")
                nc.tensor.matmul(
                    maxb_ps[:], ten32[:], m1row[:], start=True, stop=True
                )
                maxb = auxpool.tile([128, QCH], F32, tag="maxb")
                nc.vector.tensor_copy(maxb[:], maxb_ps[:])
                return maxb

            def stt_exp(st, maxb, p_t, j, rsum):
                # exp(10*s - 10*max) for k-tiles 4j..4j+3; one wide exp
                # amortizes the ~185ns activation launch overhead. The
                # otherwise-idle gpsimd engine accumulates the running
                # per-query sum of P behind each exp batch.
                for kt in range(4 * j, 4 * j + 4):
                    nc.vector.scalar_tensor_tensor(
                        st[:, kt, :],
                        st[:, kt, :],
                        10.0,
                        maxb[:],
                        op0=AL.mult,
                        op1=AL.subtract,
                    )
                nc.scalar.activation(
                    p_t[:, 4 * j : 4 * j + 4, :],
                    st[:, 4 * j : 4 * j + 4, :],
                    EXP,
                )
                if rsum is None:
                    return
                for kt in range(4 * j, 4 * j + 4):
                    if kt == 0:
                        continue
                    if kt == 1:
                        nc.gpsimd.tensor_add(
                            rsum[:], p_t[:, 0, :], p_t[:, 1, :]
                        )
                    else:
                        nc.gpsimd.tensor_add(rsum[:], rsum[:], p_t[:, kt, :])

            def pv_tiles(c, p_t, dtis):
                q0 = QCH * c
                for dti in dtis:
                    d0 = 128 * dti
                    ops = psO.tile([128, QCH], F32, tag="psO")
                    for kt in range(KT):
                        nc.tensor.matmul(
                            ops[:],
                            vf[:, kt, d0 : d0 + 128],
                            p_t[:, kt, :],
                            start=(kt == 0),
                            stop=(kt == KT - 1),
                        )
                    osb = outpool.tile([128, QCH], F16, tag="osb")
                    nc.vector.tensor_copy(osb[:], ops[:])
                    nc.scalar.dma_start(ot_r[:, dti, q0 : q0 + QCH], osb[:])

            def sum_row(c, rsum):
                # finish the per-query sum across the 128 partitions with a
                # gpsimd all-reduce; row 0 ships to host for the 1/sum
                # normalization. Never touches the tensor or vector engines;
                # nothing downstream depends on it.
                sall = auxpool.tile([128, QCH], F32, tag="sall")
                nc.gpsimd.partition_all_reduce(
                    sall[:], rsum[:], 128, bass_isa.ReduceOp.add
                )
                nc.gpsimd.dma_start(sm_d.ap()[c : c + 1, :], sall[0:1, :])

            # unrolled schedule (NCH == 2): each chunk's softmax (STT/exp) is
            # interleaved at fine grain into the next tensor tile stream
            # (chunk 1's QK / chunk 0's PV) so no engine FIFO ever blocks on
            # a cross-engine dependency
            st0 = stpool.tile([128, KT, QCH], F32, tag="st")
            rmax0 = auxpool.tile([128, QCH], F32, tag="rmax")
            qk_tiles(0, st0, range(KT), rmax0)
            m1row0 = fold_max(rmax0)
            st1 = stpool.tile([128, KT, QCH], F32, tag="st")
            rmax1 = auxpool.tile([128, QCH], F32, tag="rmax")
            qk_tiles(1, st1, range(0, 5), rmax1)
            maxb0 = bcast_max(m1row0)
            p0 = ppool.tile([128, KT, QCH], F16, tag="p")
            rsum0 = auxpool.tile([128, QCH], F32, tag="rsum")
            for j in range(4):
                qk_tiles(1, st1, range(5 + 2 * j, 7 + 2 * j), rmax1)
                stt_exp(st0, maxb0, p0, j, rsum0)
            qk_tiles(1, st1, range(13, KT), rmax1)
            m1row1 = fold_max(rmax1)
            pv_tiles(0, p0, range(0, 3))
            sum_row(0, rsum0)
            maxb1 = bcast_max(m1row1)
            p1 = ppool.tile([128, KT, QCH], F16, tag="p")
            for j in range(4):
                stt_exp(st1, maxb1, p1, j, None)
                pv_tiles(0, p0, range(3 + j, 4 + j))
            pv_tiles(0, p0, range(7, DT))
            # chunk 1's sums run on the vector engine (idle during PV) so
            # the serial gpsimd add chain doesn't extend the kernel tail
            rsum1 = auxpool.tile([128, QCH], F32, tag="rsum")
            nc.vector.tensor_add(rsum1[:], p1[:, 0, :], p1[:, 1, :])
            for kt in range(2, KT):
                nc.vector.tensor_add(rsum1[:], rsum1[:], p1[:, kt, :])
            sum_row(1, rsum1)
            pv_tiles(1, p1, range(DT))

    nc.compile()
    _BUILT["nc"] = nc
    return nc


def _prep_inputs(x, q_w, k_w, v_w):
    f16 = np.float16
    g = np.ascontiguousarray(q_w.T @ k_w).astype(f16)
    wv = np.ascontiguousarray(v_w.T).astype(f16)

    in_maps = []
    xk_cache = {}
    for core in range(NCORES):
        b, h = divmod(core, 2)
        if b not in xk_cache:
            xk_cache[b] = np.ascontiguousarray(np.asarray(x[b]).T).astype(f16)
        xk = xk_cache[b]
        xq = np.ascontiguousarray(xk[:, NQ * h : NQ * (h + 1)])
        in_maps.append({"xq": xq, "xk": xk, "g": g, "wv": wv})
    return in_maps


def run(x, q_w, k_w, v_w, trace=False):
    from concourse.bass_utils import run_bass_kernel_spmd

    nc = _build()
    in_maps = _prep_inputs(x, q_w, k_w, v_w)
    res = run_bass_kernel_spmd(nc, in_maps, list(range(NCORES)), trace=trace)
    out = np.empty((B, SEQ, D), np.float32)
    for core in range(NCORES):
        b, h = divmod(core, 2)
        ot = res.results[core]["ot"].T.astype(np.float32)
        sm = res.results[core]["sm"].reshape(NQ).astype(np.float32)
        out[b, NQ * h : NQ * (h + 1)] = ot / sm[:, None]
    return out, res


def kernel(x, q_w, k_w, v_w):
    x = np.asarray(x, np.float32)
    q_w = np.asarray(q_w, np.float32)
    k_w = np.asarray(k_w, np.float32)
    v_w = np.asarray(v_w, np.float32)
    out, _ = run(x, q_w, k_w, v_w, trace=False)
    return out


# revision 37
# speedup vs baseline: 1.1227x; 1.1021x over previous
"""Single-head attention (B=4, N=2048, D=1024), scores scaled by 10.

Sharding: 8 cores = (batch, query-half). Core 2b+h owns queries
[1024h:1024(h+1)] of batch b.

Algebraic restructure: scores = Q K^T = x_q (Wq^T Wk) x_k^T, so
G = q_w^T @ k_w is precomputed on host and the kernel computes
U = x_q G on device — the K projection disappears and the key side of
QK^T is the raw x, which every core receives in full from the host
(no K collective). Only V halves are exchanged, via two pair
AllGathers (one per 512-wide e-half) so the first exchange overlaps
the second half of the V projection.

Numerics: everything runs single-pass fp16 with fp32 PSUM
accumulation (measured end-to-end rel err ~4.6e-3 vs the 2e-2 gate;
the x10 score scale makes bf16 single-pass fail, but fp16's 10-bit
mantissa keeps the softmax exponent error ~0.04). The 1/sum
normalization happens on the HOST: the kernel emits unnormalized O^T
plus a per-query sum row, removing the reciprocal broadcast from the
critical path.

Schedule (fully unrolled, two 512-query chunks): scores stay
k-partitioned (st tiles) so attention@V consumes P with no
transposes. The tensor queue is just proj, QK(0), QK(1), PV(0),
PV(1) plus one rank-1 max-broadcast matmul per chunk. A running
per-query max rides the QK copy stream (one DVE max per k-tile, zero
added latency), is folded 128->32 partitions with 4 small DMAs +
32x32 block transposes to a [1, QCH] row, and each chunk's scale+exp
is interleaved at 2-k-tile / 1-d-tile granularity into the NEXT
tensor block so no engine FIFO ever blocks cross-engine. Per-query
sums: running adds on the otherwise-idle gpsimd engine (chunk 0) /
on vector during PV (chunk 1), each finished by one gpsimd
partition_all_reduce — sums never touch the tensor engine. Queue
discipline: input loads own the sync ring; V staging and output
writes ride the scalar ring; collectives, V readbacks, partition
reductions, and the sum-row writes stay on the gpsimd queue (a DMA
queued behind a foreign-dependency DMA on the same ring inherits its
wait — keep unrelated consumers on separate rings).
"""

import numpy as np

B, SEQ, D = 4, 2048, 1024
NQ = 1024          # queries per core (= keys computed per core)
QCH = 512          # attention q-chunk
NCH = NQ // QCH    # 2
NCORES = 8
DT = D // 128      # 8 d-tiles
KT = SEQ // 128    # 16 k-tiles
HKT = KT // 2      # 8 own-half k-tiles

_BUILT = {}


def _build():
    if "nc" in _BUILT:
        return _BUILT["nc"]
    from contextlib import ExitStack

    import concourse.bass as bass  # noqa: F401
    import concourse.mybir as mybir
    import concourse.tile as tile
    from concourse import bacc, bass_isa

    dt = mybir.dt
    F32, BF, F16 = dt.float32, dt.bfloat16, dt.float16
    AL = mybir.AluOpType
    EXP = mybir.ActivationFunctionType.Exp
    GROUPS = [[2 * i, 2 * i + 1] for i in range(NCORES // 2)]

    nc = bacc.Bacc("TRN2", target_bir_lowering=False, debug=False)

    xq_d = nc.dram_tensor("xq", [D, NQ], F16, kind="ExternalInput")
    xk_d = nc.dram_tensor("xk", [D, SEQ], F16, kind="ExternalInput")
    g_d = nc.dram_tensor("g", [D, D], F16, kind="ExternalInput")
    wv_d = nc.dram_tensor("wv", [D, D], F16, kind="ExternalInput")
    ot_d = nc.dram_tensor("ot", [D, NQ], F16, kind="ExternalOutput")
    sm_d = nc.dram_tensor("sm", [NCH, QCH], F32, kind="ExternalOutput")

    xq_r = xq_d.ap().rearrange("(t p) n -> p t n", p=128)
    xk_r = xk_d.ap().rearrange("(t p) n -> p t n", p=128)
    g_r = g_d.ap().rearrange("(t p) e -> p t e", p=128)
    wv_r = wv_d.ap().rearrange("(t p) e -> p t e", p=128)
    ot_r = ot_d.ap().rearrange("(t p) q -> p t q", p=128)

    with tile.TileContext(nc) as tc, ExitStack() as ctx:
        main_pool = ctx.enter_context(tc.tile_pool(name="main", bufs=1))
        xk_t = main_pool.tile([128, DT, SEQ], F16, tag="xk")
        uth = main_pool.tile([128, DT, NQ], F16, tag="uth")
        vf = main_pool.tile([128, KT, D], F16, tag="vf")

        const_pool = ctx.enter_context(tc.tile_pool(name="const", bufs=1))
        ten32 = const_pool.tile([1, 128], F32, tag="ten32")
        nc.vector.memset(ten32[:], 10.0)

        dram = ctx.enter_context(tc.tile_pool(name="dram", bufs=1, space="DRAM"))
        v_in0 = dram.tile([NQ, 512], F16, tag="v_in0")
        v_out0 = dram.tile([SEQ, 512], F16, tag="v_out0")
        v_in1 = dram.tile([NQ, 512], F16, tag="v_in1")
        v_out1 = dram.tile([SEQ, 512], F16, tag="v_out1")
        warm_in = dram.tile([16, 16], BF, tag="warm_in")
        warm_out = dram.tile([32, 16], BF, tag="warm_out")

        # tiny warmup collective at t=0: pays the ncfw channel-setup latency
        # before the real V exchanges need it
        warm_sb = const_pool.tile([16, 16], BF, tag="warm_sb")
        nc.vector.memset(warm_sb[:], 0.0)
        nc.sync.dma_start(warm_in[:], warm_sb[:])
        nc.gpsimd.collective_compute(
            "AllGather",
            AL.bypass,
            replica_groups=GROUPS,
            ins=[warm_in[:]],
            outs=[warm_out[:]],
        )

        with (
            tc.tile_pool(name="xqp", bufs=1) as xqp,
            tc.tile_pool(name="gw", bufs=1) as gwpool,
            tc.tile_pool(name="kev", bufs=6) as kevpool,
            tc.tile_pool(name="psA", bufs=4, space="PSUM") as psA,
        ):
            xq_t = xqp.tile([128, DT, NQ], F16, tag="xq")
            wv_t = gwpool.tile([128, DT, D], F16, tag="wv")
            g_t = gwpool.tile([128, DT, D], F16, tag="g")
            # input loads in first-use order, all on the sync ring:
            # U-proj chunk 0 needs g h0 + xq h0 (2MB) — tensor starts ~+12us
            nc.sync.dma_start(g_t[:, :, 0:512], g_r[:, :, 0:512])
            for dti in range(DT):
                nc.sync.dma_start(xq_t[:, dti, 0:512], xq_r[:, dti, 0:512])
            nc.sync.dma_start(g_t[:, :, 512:1024], g_r[:, :, 512:1024])
            nc.sync.dma_start(wv_t[:, :, 0:512], wv_r[:, :, 0:512])
            for dti in range(DT):
                nc.sync.dma_start(xq_t[:, dti, 512:1024], xq_r[:, dti, 512:1024])
            nc.sync.dma_start(wv_t[:, :, 512:1024], wv_r[:, :, 512:1024])
            # keys (full x) only needed by attention QK^T — load last
            for j in range(4):
                n0 = 512 * j
                nc.sync.dma_start(xk_t[:, :, n0 : n0 + 512], xk_r[:, :, n0 : n0 + 512])

            def u_proj(chn):
                n0 = 512 * chn
                for et in range(DT):
                    e0 = 128 * et
                    ps = psA.tile([128, 512], F32, tag="psA")
                    for dti in range(DT):
                        nc.tensor.matmul(
                            ps[:],
                            g_t[:, dti, e0 : e0 + 128],
                            xq_t[:, dti, n0 : n0 + 512],
                            start=(dti == 0),
                            stop=(dti == DT - 1),
                        )
                    nc.vector.tensor_copy(uth[:, et, n0 : n0 + 512], ps[:])

            def v_proj(ec, v_in):
                e0 = 512 * ec
                for kt in range(HKT):
                    k0 = 128 * kt
                    ps = psA.tile([128, 512], F32, tag="psA")
                    for dti in range(DT):
                        nc.tensor.matmul(
                            ps[:],
                            xq_t[:, dti, k0 : k0 + 128],
                            wv_t[:, dti, e0 : e0 + 512],
                            start=(dti == 0),
                            stop=(dti == DT - 1),
                        )
                    vev = kevpool.tile([128, 512], F16, tag="vev")
                    nc.vector.tensor_copy(vev[:], ps[:])
                    nc.scalar.dma_start(v_in[k0 : k0 + 128, :], vev[:])

            # ---- U^T chunk 0, then V halves (each launching its exchange),
            # ---- then U^T chunk 1
            u_proj(0)
            for ec, (v_in, v_out) in enumerate(((v_in0, v_out0), (v_in1, v_out1))):
                v_proj(ec, v_in)
                nc.gpsimd.collective_compute(
                    "AllGather",
                    AL.bypass,
                    replica_groups=GROUPS,
                    ins=[v_in[:]],
                    outs=[v_out[:]],
                )
                v_out_r = v_out[:].rearrange("(t p) e -> p t e", p=128)
                e0 = 512 * ec
                for j in range(2):
                    t0 = 8 * j
                    nc.gpsimd.dma_start(
                        vf[:, t0 : t0 + 8, e0 : e0 + 512], v_out_r[:, t0 : t0 + 8, :]
                    )
            u_proj(1)

        # ---------------- Phase B: attention, two 512-query chunks --------
        with (
            tc.tile_pool(name="stp", bufs=2) as stpool,
            tc.tile_pool(name="pp", bufs=2) as ppool,
            tc.tile_pool(name="tree", bufs=1) as treepool,
            tc.tile_pool(name="aux", bufs=2) as auxpool,
            tc.tile_pool(name="osb", bufs=3) as outpool,
            tc.tile_pool(name="psS", bufs=4, space="PSUM") as psS,
            tc.tile_pool(name="psO", bufs=3, space="PSUM") as psO,
            tc.tile_pool(name="psX", bufs=1, space="PSUM") as psX,
        ):
            def qk_tiles(c, st, kts, rmax):
                # QK^T for the given k-tiles; a running per-query max is
                # folded into the copy stream (one DVE max per tile, zero
                # added latency vs a post-hoc reduction)
                q0 = QCH * c
                for kt in kts:
                    k0 = 128 * kt
                    ps = psS.tile([128, QCH], F32, tag="psS")
                    for dti in range(DT):
                        nc.tensor.matmul(
                            ps[:],
                            xk_t[:, dti, k0 : k0 + 128],
                            uth[:, dti, q0 : q0 + QCH],
                            start=(dti == 0),
                            stop=(dti == DT - 1),
                        )
                    nc.vector.tensor_copy(st[:, kt, :], ps[:])
                    if kt == 1:
                        nc.vector.tensor_max(rmax[:], st[:, 0, :], st[:, 1, :])
                    elif kt >= 2:
                        nc.vector.tensor_max(rmax[:], rmax[:], st[:, kt, :])

            def fold_max(rmax):
                # fold the running per-partition max 128 -> 32 partitions
                # (DVE ops need equal start partitions, so move 32-partition
                # groups with DMAs), then 32x32 block transposes down to a
                # [1, QCH] max row
                fold4 = treepool.tile([32, 4, QCH], F32, tag="fold4")
                for a in range(4):
                    nc.sync.dma_start(
                        fold4[:, a, :], rmax[32 * a : 32 * (a + 1), :]
                    )
                nc.vector.tensor_max(fold4[:, 0, :], fold4[:, 0, :], fold4[:, 1, :])
                nc.vector.tensor_max(fold4[:, 2, :], fold4[:, 2, :], fold4[:, 3, :])
                nc.vector.tensor_max(fold4[:, 0, :], fold4[:, 0, :], fold4[:, 2, :])
                t32t = treepool.tile([32, QCH], F32, tag="t32t")
                nc.vector.transpose(t32t[:], fold4[:, 0, :])
                mx32 = treepool.tile([32, 32], F32, tag="mx32")
                nc.vector.memset(mx32[:], 0.0)
                nc.vector.reduce_max(
                    mx32[:, 0 : QCH // 32],
                    t32t[:].rearrange("p (j c) -> p j c", c=32),
                    axis=mybir.AxisListType.X,
                )
                mx32t = treepool.tile([32, 32], F32, tag="mx32t")
                nc.vector.transpose(mx32t[:], mx32[:])
                m1row = treepool.tile([1, QCH], F32, tag="m1row")
                nc.sync.dma_start(m1row[:], mx32t[0 : QCH // 32, :])
                return m1row

            def bcast_max(m1row):
                # broadcast 10*max across partitions with a rank-1 matmul
                maxb_ps = psX.tile([128, QCH], F32, tag="bcast")
                nc.tensor.matmul(
                    maxb_ps[:], ten32[:], m1row[:], start=True, stop=True
                )
                maxb = auxpool.tile([128, QCH], F32, tag="maxb")
                nc.vector.tensor_copy(maxb[:], maxb_ps[:])
                return maxb

            def stt_exp(st, maxb, p_t, j, rsum):
                # exp(10*s - 10*max) for k-tiles 4j..4j+3; one wide exp
                # amortizes the ~185ns activation launch overhead. The
                # otherwise-idle gpsimd engine accumulates the running
                # per-query sum of P behind each exp batch.
                for kt in range(4 * j, 4 * j + 4):
                    nc.vector.scalar_tensor_tensor(
                        st[:, kt, :],
                        st[:, kt, :],
                        10.0,
                        maxb[:],
                        op0=AL.mult,
                        op1=AL.subtract,
                    )
                nc.scalar.activation(
                    p_t[:, 4 * j : 4 * j + 4, :],
                    st[:, 4 * j : 4 * j + 4, :],
                    EXP,
                )
                if rsum is None:
                    return
                for kt in range(4 * j, 4 * j + 4):
                    if kt == 0:
                        continue
                    if kt == 1:
                        nc.gpsimd.tensor_add(
                            rsum[:], p_t[:, 0, :], p_t[:, 1, :]
                        )
                    else:
                        nc.gpsimd.tensor_add(rsum[:], rsum[:], p_t[:, kt, :])

            def pv_tiles(c, p_t, dtis):
                q0 = QCH * c
                for dti in dtis:
                    d0 = 128 * dti
                    ops = psO.tile([128, QCH], F32, tag="psO")
                    for kt in range(KT):
                        nc.tensor.matmul(
                            ops[:],
                            vf[:, kt, d0 : d0 + 128],
                            p_t[:, kt, :],
                            start=(kt == 0),
                            stop=(kt == KT - 1),
                        )
                    osb = outpool.tile([128, QCH], F16, tag="osb")
                    nc.vector.tensor_copy(osb[:], ops[:])
                    nc.scalar.dma_start(ot_r[:, dti, q0 : q0 + QCH], osb[:])

            def sum_row(c, rsum):
                # finish the per-query sum across the 128 partitions with a
                # gpsimd all-reduce; row 0 ships to host for the 1/sum
                # normalization. Never touches the tensor or vector engines;
                # nothing downstream depends on it.
                sall = auxpool.tile([128, QCH], F32, tag="sall")
                nc.gpsimd.partition_all_reduce(
                    sall[:], rsum[:], 128, bass_isa.ReduceOp.add
                )
                nc.gpsimd.dma_start(sm_d.ap()[c : c + 1, :], sall[0:1, :])

            # unrolled schedule (NCH == 2): each chunk's softmax (STT/exp) is
            # interleaved at fine grain into the next tensor tile stream
            # (chunk 1's QK / chunk 0's PV) so no engine FIFO ever blocks on
            # a cross-engine dependency
            st0 = stpool.tile([128, KT, QCH], F32, tag="st")
            rmax0 = auxpool.tile([128, QCH], F32, tag="rmax")
            qk_tiles(0, st0, range(KT), rmax0)
            m1row0 = fold_max(rmax0)
            st1 = stpool.tile([128, KT, QCH], F32, tag="st")
            rmax1 = auxpool.tile([128, QCH], F32, tag="rmax")
            qk_tiles(1, st1, range(0, 6), rmax1)
            maxb0 = bcast_max(m1row0)
            p0 = ppool.tile([128, KT, QCH], F16, tag="p")
            rsum0 = auxpool.tile([128, QCH], F32, tag="rsum")
            for j in range(4):
                qk_tiles(1, st1, range(6 + 2 * j, 8 + 2 * j), rmax1)
                stt_exp(st0, maxb0, p0, j, rsum0)
            qk_tiles(1, st1, range(14, KT), rmax1)
            m1row1 = fold_max(rmax1)
            pv_tiles(0, p0, range(0, 3))
            sum_row(0, rsum0)
            maxb1 = bcast_max(m1row1)
            p1 = ppool.tile([128, KT, QCH], F16, tag="p")
            # last two STT/exp batches go ahead of the osb-heavy PV0 tail so
            # exp(1) clears the vector FIFO before PV(1) needs p1
            stt_exp(st1, maxb1, p1, 0, None)
            pv_tiles(0, p0, range(3, 4))
            stt_exp(st1, maxb1, p1, 1, None)
            pv_tiles(0, p0, range(4, 5))
            stt_exp(st1, maxb1, p1, 2, None)
            stt_exp(st1, maxb1, p1, 3, None)
            pv_tiles(0, p0, range(5, DT))
            # chunk 1's sums run on the vector engine (idle during PV) so
            # the serial gpsimd add chain doesn't extend the kernel tail
            rsum1 = auxpool.tile([128, QCH], F32, tag="rsum")
            nc.vector.tensor_add(rsum1[:], p1[:, 0, :], p1[:, 1, :])
            for kt in range(2, KT):
                nc.vector.tensor_add(rsum1[:], rsum1[:], p1[:, kt, :])
            sum_row(1, rsum1)
            pv_tiles(1, p1, range(DT))

    nc.compile()
    _BUILT["nc"] = nc
    return nc


def _prep_inputs(x, q_w, k_w, v_w):
    f16 = np.float16
    g = np.ascontiguousarray(q_w.T @ k_w).astype(f16)
    wv = np.ascontiguousarray(v_w.T).astype(f16)

    in_maps = []
    xk_cache = {}
    for core in range(NCORES):
        b, h = divmod(core, 2)
        if b not in xk_cache:
            xk_cache[b] = np.ascontiguousarray(np.asarray(x[b]).T).astype(f16)
        xk = xk_cache[b]
        xq = np.ascontiguousarray(xk[:, NQ * h : NQ * (h + 1)])
        in_maps.append({"xq": xq, "xk": xk, "g": g, "wv": wv})
    return in_maps


def run(x, q_w, k_w, v_w, trace=False):
    from concourse.bass_utils import run_bass_kernel_spmd

    nc = _build()
    in_maps = _prep_inputs(x, q_w, k_w, v_w)
    res = run_bass_kernel_spmd(nc, in_maps, list(range(NCORES)), trace=trace)
    out = np.empty((B, SEQ, D), np.float32)
    for core in range(NCORES):
        b, h = divmod(core, 2)
        ot = res.results[core]["ot"].T.astype(np.float32)
        sm = res.results[core]["sm"].reshape(NQ).astype(np.float32)
        out[b, NQ * h : NQ * (h + 1)] = ot / sm[:, None]
    return out, res


def kernel(x, q_w, k_w, v_w):
    x = np.asarray(x, np.float32)
    q_w = np.asarray(q_w, np.float32)
    k_w = np.asarray(k_w, np.float32)
    v_w = np.asarray(v_w, np.float32)
    out, _ = run(x, q_w, k_w, v_w, trace=False)
    return out
